# revision 1
# baseline (speedup 1.0000x reference)
"""Bivariate Gaussian kernel (Nadaraya-Watson) on 8 TRN2 NeuronCores.

Math: for query m, result[m] = t[m] / (s[m] + EPS) where
  w[n,m] = exp(-c[m] * d2[n,m]),  c[m] = 1/(2*bw[m]^2)
  s[m] = sum_n w[n,m],  t[m] = sum_n w[n,m]*outputs[n]

Device algorithm (per core, M_loc=1024 queries):
  exponent E[n,m] = P[m] + Q[m]*a2[n] + R[m]*in0[n] + S[m]*in1[n]
    (P=-c*b2, Q=-c, R=2c*x0, S=2c*x1) computed as rank-11 matmuls on the PE
    using error-compensated bf16 hi/lo splits (~1e-3 abs exact),
    with 3-4 n-tiles packed CONCURRENTLY into the 128x128 array via row
    tile_position (k=11 strips at rows 0/32/64/96 run simultaneously),
  W = exp(E) on the scalar engine (PSUM -> SBUF, bf16),
  [s; t_hi; t_lo] accumulated over n by a second matmul with stationary
    [ones, out_hi, out_lo] (bf16), PSUM accumulation across all 64 n-tiles;
    both m-half accumulators share one PSUM bank via col tile_position.
Queries (M) are sharded across the 8 cores; each core sees all N points.
"""

import functools
import sys

import numpy as np

sys.path.insert(0, "/opt/trn_rl_repo")

EPS = 1e-7
N = 8192
M = 8192
NCORES = 8
MLOC = M // NCORES  # 1024
P = 128
NT = N // P  # 64 n-tiles
MBW = 512  # m-block width (one PSUM bank)
MB = MLOC // MBW  # 2 m-blocks
NBLK = NT * MB  # 128 blocks of (128n x 512m)
K = 11  # compensated-split rank

# per-half n-tile grouping: alternates the 4-bank and 3-bank PSUM buffers
HALF_SIZES = [1, 2] + [4, 3] * 8 + [4, 1]
assert sum(HALF_SIZES) == NT and len(HALF_SIZES) % 2 == 0


def _half_groups():
    out = []
    pos = 0
    for sz in HALF_SIZES:
        out.append(list(range(pos, pos + sz)))
        pos += sz
    return out


@functools.lru_cache(maxsize=1)
def _build():
    import concourse.tile as tile
    from concourse import bacc, mybir

    f32 = mybir.dt.float32
    bf16 = mybir.dt.bfloat16
    EXP = mybir.ActivationFunctionType.Exp

    tgroups = _half_groups()
    NG = len(tgroups)  # col-slices in the packed stationary

    nc = bacc.Bacc("TRN2", target_bir_lowering=False, debug=False, num_devices=NCORES)
    # packed E stationary: band r (rows 32r..32r+10) of col-slice g holds the
    # A-rows of the r-th n-tile of group g. Rows outside the bands are unread.
    stat_d = nc.dram_tensor("stat", [P, NG * P], bf16, kind="ExternalInput")
    # E moving: every band holds the same 11 B-rows (PQRS hi/lo splits).
    mov_d = nc.dram_tensor("mov", [P, MLOC], bf16, kind="ExternalInput")
    rsb_d = nc.dram_tensor("rsb", [P, 4 * NT], bf16, kind="ExternalInput")
    res_d = nc.dram_tensor("res", [3 * MB, MBW], f32, kind="ExternalOutput")

    with tile.TileContext(nc) as tc:
        with (
            tc.tile_pool(name="const", bufs=1) as cpool,
            tc.tile_pool(name="w", bufs=5) as wpool,
            tc.tile_pool(name="epsum", bufs=1, space="PSUM") as epool,
            tc.tile_pool(name="acc", bufs=1, space="PSUM") as apool,
        ):
            # PE warm-up + exp-table preload on a never-written (garbage)
            # tile: no data deps, so both start right after the preamble and
            # run while the input DMAs stream. Results are never read.
            junk = cpool.tile([P, MBW], bf16, tag="junk")
            nc.gpsimd.memset(junk[0:1, 0:1], 0.0)
            ed = epool.tile([P, MBW * 4], f32, tag="e4")
            for _ in range(2):
                nc.tensor.matmul(
                    ed[:, 0:MBW], junk[:, 0:P], junk[:], start=True, stop=True
                )

            # input loads, split across the gpsimd and scalar DMA queues;
            stat = cpool.tile([P, NG * P], bf16)
            mov = cpool.tile([P, MLOC], bf16)
            rsb = cpool.tile([P, 4 * NT], bf16)
            # full-image transfers (128-partition DMAs use all SBUF ports),
            # chunked so the first groups' data lands as early as possible.
            nc.sync.dma_start(mov[:, 0:MBW], mov_d[:, 0:MBW])
            nc.sync.dma_start(stat[:, 0 : 2 * P], stat_d[:, 0 : 2 * P])
            nc.sync.dma_start(rsb[:], rsb_d[:])
            nc.scalar.dma_start(mov[:, MBW:MLOC], mov_d[:, MBW:MLOC])
            # exp-table preload on garbage input; result never read
            scr2 = cpool.tile([1, 8], f32, tag="scr2")
            nc.scalar.activation(scr2[:], junk[0:1, 0:8], EXP)
            off = 2
            for cw in [4, 7, 7]:
                nc.gpsimd.dma_start(
                    stat[:, off * P : (off + cw) * P],
                    stat_d[:, off * P : (off + cw) * P],
                )
                off += cw

            # both m-half accumulators share one PSUM bank: rows [s;t_hi;t_lo]
            # at partitions 0-2 (m-lo) and 32-34 (m-hi, via col tile_position).
            acc = apool.tile([35, MBW], f32)

            started = [False] * MB
            pending = []

            def evict(h):
                st = cpool.tile([3, MBW], f32, tag=f"st{h}")
                nc.vector.tensor_copy(st[:], acc[32 * h : 32 * h + 3, :])
                nc.gpsimd.dma_start(res_d[3 * h : 3 * h + 3, :], st[:])

            def emit_reduce(w, h, tiles):
                for j, i in enumerate(tiles):
                    nc.tensor.matmul(
                        acc[32 * h : 32 * h + 3, :],
                        rsb[:, 4 * i : 4 * i + 3],
                        w[:, j * MBW : (j + 1) * MBW],
                        start=not started[h],
                        stop=i == NT - 1,
                        tile_position=(0, 32 * h),
                    )
                    started[h] = True
                if tiles[-1] == NT - 1:
                    evict(h)

            gi = 0
            for h in range(MB):
                for g, tiles in enumerate(tgroups):
                    if gi % 2 == 0:
                        e = epool.tile([P, MBW * 4], f32, tag="e4")
                    else:
                        e = epool.tile([P, MBW * 3], f32, tag="e3")
                    gi += 1
                    # packed concurrent E matmuls: strip r computes n-tile
                    # tiles[r] using array rows 32r..32r+10.
                    for r, i in enumerate(tiles):
                        nc.tensor.matmul(
                            e[:, r * MBW : (r + 1) * MBW],
                            stat[32 * r : 32 * r + K, g * P : (g + 1) * P],
                            mov[32 * r : 32 * r + K, h * MBW : (h + 1) * MBW],
                            start=True,
                            stop=True,
                            tile_position=(32 * r, 0),
                        )
                    w = wpool.tile([P, MBW * 4], bf16, tag="w")
                    fs = len(tiles) * MBW
                    nc.scalar.activation(w[:, :fs], e[:, :fs], EXP)
                    pending.append((w, h, tiles))
                    if len(pending) > 3:
                        emit_reduce(*pending.pop(0))
            for args in pending:
                emit_reduce(*args)

    nc.compile()
    return nc


def _bf16_split(v):
    import ml_dtypes

    hi = v.astype(ml_dtypes.bfloat16)
    lo = (v - hi.astype(np.float64)).astype(ml_dtypes.bfloat16)
    return hi, lo


def _prepare(x, inputs, outputs, bandwidth):
    """Host-side O(N+M) prep of the factored operands."""
    import ml_dtypes

    in0 = inputs[:, 0].astype(np.float64)
    in1 = inputs[:, 1].astype(np.float64)
    a2 = in0 * in0 + in1 * in1
    x0 = x[:, 0].astype(np.float64)
    x1 = x[:, 1].astype(np.float64)
    b2 = x0 * x0 + x1 * x1
    c = 1.0 / (2.0 * bandwidth.astype(np.float64) ** 2)
    Pm = -c * b2
    Qm = -c
    Rm = 2.0 * c * x0
    Sm = 2.0 * c * x1

    ones = np.ones(N, np.float64)
    a2h, a2l = _bf16_split(a2)
    i0h, i0l = _bf16_split(in0)
    i1h, i1l = _bf16_split(in1)
    oneh, _ = _bf16_split(ones)
    Ph, Pl = _bf16_split(Pm)
    Qh, Ql = _bf16_split(Qm)
    Rh, Rl = _bf16_split(Rm)
    Sh, Sl = _bf16_split(Sm)

    # row pairing: E = P(hi+lo) + a2hi*Q(hi+lo) + a2lo*Qhi + (same for in0,in1)
    stat_rows = np.stack(
        [oneh, oneh, a2h, a2h, a2l, i0h, i0h, i0l, i1h, i1h, i1l]
    )  # (K, N)
    mov_rows = np.stack([Ph, Pl, Qh, Ql, Qh, Rh, Rl, Rh, Sh, Sl, Sh])  # (K, M)

    tgroups = _half_groups()
    NG = len(tgroups)
    stat = np.zeros((P, NG * P), ml_dtypes.bfloat16)
    for g, tiles in enumerate(tgroups):
        for r, i in enumerate(tiles):
            stat[32 * r : 32 * r + K, g * P : (g + 1) * P] = stat_rows[
                :, i * P : (i + 1) * P
            ]
    mov = np.zeros((P, M), ml_dtypes.bfloat16)
    for r in range(4):
        mov[32 * r : 32 * r + K, :] = mov_rows

    oh, ol = _bf16_split(outputs.astype(np.float64))
    rsb = np.zeros((N, 4), ml_dtypes.bfloat16)
    rsb[:, 0] = 1.0
    rsb[:, 1] = oh
    rsb[:, 2] = ol
    # per n-tile lhsT layout: rsb_sb[p, 4i+c] = rsb[i*128+p, c]
    rsb_sb = np.ascontiguousarray(
        rsb.reshape(NT, P, 4).transpose(1, 0, 2).reshape(P, 4 * NT)
    )
    return stat, mov, rsb_sb


def kernel(x, inputs, outputs, bandwidth):
    from concourse.bass_utils import run_bass_kernel_spmd

    x = np.asarray(x, np.float32)
    inputs = np.asarray(inputs, np.float32)
    outputs = np.asarray(outputs, np.float32)
    bandwidth = np.asarray(bandwidth, np.float32)

    stat, mov, rsb_sb = _prepare(x, inputs, outputs, bandwidth)

    nc = _build()
    in_maps = [
        {
            "stat": stat,
            "mov": np.ascontiguousarray(mov[:, c * MLOC : (c + 1) * MLOC]),
            "rsb": rsb_sb,
        }
        for c in range(NCORES)
    ]
    try:
        res = run_bass_kernel_spmd(nc, in_maps, list(range(NCORES)))
    except Exception:
        # transient NRT_EXEC_UNIT_UNRECOVERABLE after an interrupted prior
        # run; the device recovers after a short wait.
        import time

        time.sleep(20)
        res = run_bass_kernel_spmd(nc, in_maps, list(range(NCORES)))
    parts = []
    for c in range(NCORES):
        st = res.results[c]["res"]  # (6,512): [s,t_hi,t_lo] x {m-lo, m-hi}
        s = np.concatenate([st[0], st[3]])
        t = np.concatenate([st[1] + st[2], st[4] + st[5]])
        parts.append(t / (s + EPS))
    return np.concatenate(parts).astype(np.float32)


if __name__ == "__main__":
    rng = np.random.default_rng(0)
    x = rng.standard_normal((M, 2), np.float32)
    inputs = rng.standard_normal((N, 2), np.float32)
    outputs = rng.standard_normal(N, np.float32)
    bandwidth = (0.5 + rng.random(M)).astype(np.float32)
    got = kernel(x, inputs, outputs, bandwidth)
    print(got[:8])



# revision 5
# speedup vs baseline: 3.9356x; 3.9356x over previous
"""Bivariate Gaussian kernel (Nadaraya-Watson) on 8 TRN2 NeuronCores.

Math: for query m, result[m] = t[m] / (s[m] + EPS) where
  w[n,m] = exp(-c[m] * d2[n,m]),  c[m] = 1/(2*bw[m]^2)
  s[m] = sum_n w[n,m],  t[m] = sum_n w[n,m]*outputs[n]

Algorithm (separable quantized convolution, a fast-Gauss-transform):
the 2-D Gaussian factorizes per coordinate,
  w[n,m] = exp(-c*(i0[n]-x0[m])^2) * exp(-c*(i1[n]-x1[m])^2).
Each input coordinate is splat with linear-interpolation weights onto a
uniform g-level grid (host side, O(N)), giving grid mass C[j,k] and
output-weighted mass T[j,k].  Then
  s[m] = sum_jk C[j,k] * A[j,m] * B[k,m],   A[j,m]=exp(-c[m](v0_j-x0[m])^2)
  t[m] = sum_jk T[j,k] * A[j,m] * B[k,m],   B[k,m]=exp(-c[m](v1_k-x1[m])^2)
Bilinear splatting preserves the linear term of the exponent exactly, so
the error is O(spacing^2); g=64 gives rel err ~5e-3 (tolerance 2e-2).

Device flow per core (MLOC=1024 queries, g=64):
  1. E tables as rank-8 error-compensated bf16 matmuls
     E_A[j,m] = P0[m] + R0[m] v0_j + Q[m] v0_j^2  (and E_B with axis 1),
     content duplicated across two 64-column blocks so each table lives
     on all 128 partitions; 4 row-strips (A/B x m-half) packed into one
     concurrent PE pass via tile_position rows 0/32/64/96.
  2. ACT: Abuf = exp(E_A), Bbuf = exp(E_B)  (PSUM f32 -> SBUF bf16).
  3. One 128x128 block stationary [[C^T, Tlo^T],[0, Thi^T]] x Bbuf
     computes D = [D_s (rows 0-63); D_t (rows 64-127)] in one pass.
  4. DVE: PP = Abuf * D  (bf16).
  5. Reduce stationary (128,2) [[1,0],[0,1]] blocks -> [s; t] per m-half.
Host: r = t / (s + EPS).  Queries (M) sharded across the 8 cores.
"""

import functools
import sys

import numpy as np

sys.path.insert(0, "/opt/trn_rl_repo")

EPS = 1e-7
N = 8192
M = 8192
NCORES = 8
MLOC = M // NCORES  # 1024
P = 128
G = 64  # grid levels per axis
MH = 512  # m-half width (one PSUM bank)
K = 8  # compensated-split rank


@functools.lru_cache(maxsize=1)
def _build():
    import concourse.tile as tile
    from concourse import bacc, mybir

    f32 = mybir.dt.float32
    bf16 = mybir.dt.bfloat16
    EXP = mybir.ActivationFunctionType.Exp

    nc = bacc.Bacc("TRN2", target_bir_lowering=False, debug=False, num_devices=NCORES)
    # stationaries for the E matmuls: 4 bands of 8 rows at partition
    # offsets 0/32/64/96 (A, B, A, B); each (8,128) band holds the axis'
    # hi/lo-split level rows duplicated across the two 64-col halves.
    statE_d = nc.dram_tensor("statE", [104, P], bf16, kind="ExternalInput")
    # block stationary [[C^T, Tlo^T],[0, Thi^T]] (128x128)
    statD_d = nc.dram_tensor("statD", [P, P], bf16, kind="ExternalInput")
    # reduce stationary: col 0 = ones on rows 0-63, col 1 = ones on 64-127
    statR_d = nc.dram_tensor("statR", [P, 2], bf16, kind="ExternalInput")
    # moving per-m coefficient rows, same 4-band layout as statE
    mov_d = nc.dram_tensor("mov", [104, MLOC], bf16, kind="ExternalInput")
    res_d = nc.dram_tensor("res", [4, MH], f32, kind="ExternalOutput")

    with tile.TileContext(nc) as tc:
        with (
            tc.tile_pool(name="const", bufs=1) as cpool,
            tc.tile_pool(name="psum", bufs=1, space="PSUM") as ppool,
        ):
            # E tiles: EA0 | EB0 | EA1 | EB1, one PSUM bank each
            e = ppool.tile([P, 4 * MH], f32)
            d = ppool.tile([P, 2 * MH], f32)  # D halves, one bank each
            st = ppool.tile([34, MH], f32)  # [s;t] at rows 0-1 (h0), 32-33 (h1)

            # PE warm-up + exp-table preload on a never-written (garbage)
            # tile: no data deps, so both start right after the preamble and
            # run while the input DMAs stream. Results are never read.
            junk = cpool.tile([P, MH], bf16, tag="junk")
            nc.gpsimd.memset(junk[0:1, 0:1], 0.0)
            for _ in range(2):
                nc.tensor.matmul(
                    e[:, 0:MH], junk[:, 0:P], junk[:], start=True, stop=True
                )
            scr2 = cpool.tile([1, 8], f32, tag="scr2")
            nc.scalar.activation(scr2[:], junk[0:1, 0:8], EXP)

            # input loads spread across DMA queues
            statE = cpool.tile([104, P], bf16)
            statD = cpool.tile([P, P], bf16)
            statR = cpool.tile([P, 2], bf16)
            mov = cpool.tile([104, MLOC], bf16)
            nc.sync.dma_start(mov[:, 0:MH], mov_d[:, 0:MH])
            nc.scalar.dma_start(mov[:, MH:MLOC], mov_d[:, MH:MLOC])
            nc.gpsimd.dma_start(statE[:], statE_d[:])
            nc.gpsimd.dma_start(statD[:], statD_d[:])
            nc.gpsimd.dma_start(statR[:], statR_d[:])

            ab = cpool.tile([P, 4 * MH], bf16)  # A0 | B0 | A1 | B1 (bf16)
            pp = cpool.tile([P, 2 * MH], bf16)  # PP halves
            sto0 = cpool.tile([2, MH], f32, tag="sto0")
            sto1 = cpool.tile([2, MH], f32, tag="sto1")
            sto = [sto0, sto1]

            # 1) all four E matmuls in one concurrent PE pass
            for h in range(2):
                for ax in range(2):  # 0 = A (axis0), 1 = B (axis1)
                    r = 2 * h + ax
                    nc.tensor.matmul(
                        e[:, r * MH : (r + 1) * MH],
                        statE[32 * r : 32 * r + K, :],
                        mov[32 * r : 32 * r + K, h * MH : (h + 1) * MH],
                        start=True,
                        stop=True,
                        tile_position=(32 * r, 0),
                    )
            # 2) exp: B first so the D matmul can start earliest
            for h in range(2):
                ia, ib = 2 * h, 2 * h + 1
                nc.scalar.activation(
                    ab[:, ib * MH : (ib + 1) * MH], e[:, ib * MH : (ib + 1) * MH], EXP
                )
                nc.scalar.activation(
                    ab[:, ia * MH : (ia + 1) * MH], e[:, ia * MH : (ia + 1) * MH], EXP
                )
            for h in range(2):
                ia, ib = 2 * h, 2 * h + 1
                # 3) block-stationary convolution: D = [[C^T,Tlo^T],[0,Thi^T]]^T-matmul
                nc.tensor.matmul(
                    d[:, h * MH : (h + 1) * MH],
                    statD[:],
                    ab[:, ib * MH : (ib + 1) * MH],
                    start=True,
                    stop=True,
                )
                # 4) PP = A * D
                nc.vector.tensor_mul(
                    pp[:, h * MH : (h + 1) * MH],
                    ab[:, ia * MH : (ia + 1) * MH],
                    d[:, h * MH : (h + 1) * MH],
                )
                # 5) [s;t] = statR^T @ PP
                nc.tensor.matmul(
                    st[32 * h : 32 * h + 2, :],
                    statR[:],
                    pp[:, h * MH : (h + 1) * MH],
                    start=True,
                    stop=True,
                    tile_position=(0, 32 * h),
                )
                nc.vector.tensor_copy(sto[h][:], st[32 * h : 32 * h + 2, :])
                nc.gpsimd.dma_start(res_d[2 * h : 2 * h + 2, :], sto[h][:])

    nc.compile()
    return nc


def _bf16_split(v):
    import ml_dtypes

    hi = v.astype(ml_dtypes.bfloat16)
    lo = (v - hi.astype(np.float64)).astype(ml_dtypes.bfloat16)
    return hi, lo


def _prepare(x, inputs, outputs, bandwidth):
    """Host-side O(N + M) prep of grids, splat masses, and coefficients."""
    import ml_dtypes

    x = x.astype(np.float64)
    inputs = inputs.astype(np.float64)
    outputs = outputs.astype(np.float64)
    bw = bandwidth.astype(np.float64)

    # uniform grids + bilinear splat masses
    levels = []
    idx = []
    lam = []
    for ax in range(2):
        vals = inputs[:, ax]
        lv = np.linspace(vals.min(), vals.max(), G)
        j = np.clip(np.searchsorted(lv, vals) - 1, 0, G - 2)
        la = np.clip((vals - lv[j]) / (lv[j + 1] - lv[j]), 0.0, 1.0)
        levels.append(lv)
        idx.append(j)
        lam.append(la)
    (v0, v1), (j0, j1), (l0, l1) = levels, idx, lam
    C = np.zeros((G, G))
    T = np.zeros((G, G))
    for dj in (0, 1):
        for dk in (0, 1):
            wgt = (l0 if dj else 1 - l0) * (l1 if dk else 1 - l1)
            np.add.at(C, (j0 + dj, j1 + dk), wgt)
            np.add.at(T, (j0 + dj, j1 + dk), wgt * outputs)

    # E-table stationaries: rows [1,1,v2h,v2h,v2l,vh,vh,vl] per axis,
    # duplicated across the two 64-col halves; 4 bands (A,B,A,B).
    statE = np.zeros((104, P), ml_dtypes.bfloat16)
    for ax, v in ((0, v0), (1, v1)):
        vh, vl = _bf16_split(v)
        v2h, v2l = _bf16_split(v * v)
        one = np.ones(G, ml_dtypes.bfloat16)
        band = np.stack([one, one, v2h, v2h, v2l, vh, vh, vl])  # (8, G)
        band = np.concatenate([band, band], axis=1)  # (8, 128) duplicated
        statE[32 * ax : 32 * ax + K, :] = band
        statE[32 * (ax + 2) : 32 * (ax + 2) + K, :] = band

    # block stationary [[C^T, Tlo^T],[0, Thi^T]]
    Ch, _ = _bf16_split(C)
    Th, Tl = _bf16_split(T)
    statD = np.zeros((P, P), ml_dtypes.bfloat16)
    statD[0:G, 0:G] = Ch.T
    statD[0:G, G:P] = Tl.T
    statD[G:P, G:P] = Th.T

    statR = np.zeros((P, 2), ml_dtypes.bfloat16)
    statR[0:G, 0] = 1.0
    statR[G:P, 1] = 1.0

    # moving rows: [Ph,Pl,Qh,Ql,Qh,Rh,Rl,Rh] per axis, 4 bands (A,B,A,B)
    c = 1.0 / (2.0 * bw * bw)
    Qh, Ql = _bf16_split(-c)
    mov = np.zeros((104, M), ml_dtypes.bfloat16)
    for ax in range(2):
        xc = x[:, ax]
        Ph, Pl = _bf16_split(-c * xc * xc)
        Rh, Rl = _bf16_split(2.0 * c * xc)
        band = np.stack([Ph, Pl, Qh, Ql, Qh, Rh, Rl, Rh])  # (8, M)
        mov[32 * ax : 32 * ax + K, :] = band
        mov[32 * (ax + 2) : 32 * (ax + 2) + K, :] = band

    return statE, statD, statR, mov


def _make_inmaps(x, inputs, outputs, bandwidth):
    statE, statD, statR, mov = _prepare(x, inputs, outputs, bandwidth)
    return [
        {
            "statE": statE,
            "statD": statD,
            "statR": statR,
            "mov": np.ascontiguousarray(mov[:, c * MLOC : (c + 1) * MLOC]),
        }
        for c in range(NCORES)
    ]


def kernel(x, inputs, outputs, bandwidth):
    from concourse.bass_utils import run_bass_kernel_spmd

    x = np.asarray(x, np.float32)
    inputs = np.asarray(inputs, np.float32)
    outputs = np.asarray(outputs, np.float32)
    bandwidth = np.asarray(bandwidth, np.float32)

    in_maps = _make_inmaps(x, inputs, outputs, bandwidth)
    nc = _build()
    try:
        res = run_bass_kernel_spmd(nc, in_maps, list(range(NCORES)))
    except Exception:
        # transient NRT_EXEC_UNIT_UNRECOVERABLE after an interrupted prior
        # run; the device recovers after a short wait.
        import time

        time.sleep(20)
        res = run_bass_kernel_spmd(nc, in_maps, list(range(NCORES)))
    parts = []
    for c in range(NCORES):
        r4 = res.results[c]["res"]  # (4,512): [s_h0; t_h0; s_h1; t_h1]
        s = np.concatenate([r4[0], r4[2]])
        t = np.concatenate([r4[1], r4[3]])
        parts.append(t / (s + EPS))
    return np.concatenate(parts).astype(np.float32)


if __name__ == "__main__":
    rng = np.random.default_rng(0)
    x = rng.standard_normal((M, 2), np.float32)
    inputs = rng.standard_normal((N, 2), np.float32)
    outputs = rng.standard_normal(N, np.float32)
    bandwidth = (0.5 + rng.random(M)).astype(np.float32)
    got = kernel(x, inputs, outputs, bandwidth)
    print(got[:8])


# revision 7
# speedup vs baseline: 4.5415x; 1.1540x over previous
"""Bivariate Gaussian kernel (Nadaraya-Watson) on 8 TRN2 NeuronCores.

Math: for query m, result[m] = t[m] / (s[m] + EPS) where
  w[n,m] = exp(-c[m] * d2[n,m]),  c[m] = 1/(2*bw[m]^2)
  s[m] = sum_n w[n,m],  t[m] = sum_n w[n,m]*outputs[n]

Algorithm (separable quantized convolution, a fast-Gauss-transform):
the 2-D Gaussian factorizes per coordinate,
  w[n,m] = exp(-c*(i0[n]-x0[m])^2) * exp(-c*(i1[n]-x1[m])^2).
Each input coordinate is splat with linear-interpolation weights onto a
uniform g-level grid (host side, O(N)), giving grid mass C[j,k] and
output-weighted mass T[j,k].  Then
  s[m] = sum_jk C[j,k] * A[j,m] * B[k,m],   A[j,m]=exp(-c[m](v0_j-x0[m])^2)
  t[m] = sum_jk T[j,k] * A[j,m] * B[k,m],   B[k,m]=exp(-c[m](v1_k-x1[m])^2)
Bilinear splatting preserves the linear term of the exponent exactly, so
the error is O(spacing^2); g=64 gives rel err ~5e-3 (tolerance 2e-2).

Device flow per core (MLOC=1024 queries, g=64):
  1. E tables as rank-8 error-compensated bf16 matmuls
     E_A[j,m] = P0[m] + R0[m] v0_j + Q[m] v0_j^2  (and E_B with axis 1),
     content duplicated across two 64-column blocks so each table lives
     on all 128 partitions; 4 row-strips (A/B x m-half) packed into one
     concurrent PE pass via tile_position rows 0/32/64/96.
  2. ACT: Abuf = exp(E_A), Bbuf = exp(E_B)  (PSUM f32 -> SBUF bf16).
  3. One 128x128 block stationary [[C^T, Tlo^T],[0, Thi^T]] x Bbuf
     computes D = [D_s (rows 0-63); D_t (rows 64-127)] in one pass.
  4. DVE: PP = Abuf * D  (bf16).
  5. Reduce stationary (128,2) [[1,0],[0,1]] blocks -> [s; t] per m-half.
Host: r = t / (s + EPS).  Queries (M) sharded across the 8 cores.
"""

import functools
import sys

import numpy as np

sys.path.insert(0, "/opt/trn_rl_repo")

EPS = 1e-7
N = 8192
M = 8192
NCORES = 8
MLOC = M // NCORES  # 1024
P = 128
G = 64  # grid levels per axis
MH = 512  # m-half width (one PSUM bank)
K = 8  # compensated-split rank
# packed stat tensor columns: statD | statR | statE
SD0, SR0, SE0, SW = 0, P, P + 2, P + 2 + P


@functools.lru_cache(maxsize=1)
def _build():
    import concourse.tile as tile
    from concourse import bacc, mybir

    f32 = mybir.dt.float32
    bf16 = mybir.dt.bfloat16
    EXP = mybir.ActivationFunctionType.Exp

    nc = bacc.Bacc("TRN2", target_bir_lowering=False, debug=False, num_devices=NCORES)
    # packed stationaries (one DMA): cols 0-127 = statD (the block
    # stationary [[C^T, Tlo^T],[0, Thi^T]]), cols 128-129 = statR (reduce:
    # col 0 = ones on rows 0-63, col 1 = ones on rows 64-127), cols
    # 130-257 = statE (4 bands of 8 rows at partition offsets 0/32/64/96
    # (A,B,A,B); each band holds the axis' hi/lo-split level rows
    # duplicated across the two 64-col halves).
    stat_d = nc.dram_tensor("stat", [P, SW], bf16, kind="ExternalInput")
    # moving per-m coefficient rows, same 4-band layout as statE
    mov_d = nc.dram_tensor("mov", [104, MLOC], bf16, kind="ExternalInput")
    res_d = nc.dram_tensor("res", [4, MH], f32, kind="ExternalOutput")

    with tile.TileContext(nc) as tc:
        with (
            tc.tile_pool(name="const", bufs=1) as cpool,
            tc.tile_pool(name="psum", bufs=1, space="PSUM") as ppool,
        ):
            # E tiles: EA0 | EB0 | EA1 | EB1, one PSUM bank each
            e = ppool.tile([P, 4 * MH], f32)
            d = ppool.tile([P, 2 * MH], f32)  # D halves, one bank each
            st = ppool.tile([34, MH], f32)  # [s;t] at rows 0-1 (h0), 32-33 (h1)
            wps = ppool.tile([P, MH], f32)  # warm-up target bank

            # input loads first so the HWDGE queues start immediately
            stat = cpool.tile([P, SW], bf16)
            mov = cpool.tile([104, MLOC], bf16)
            nc.sync.dma_start(stat[:], stat_d[:])
            nc.scalar.dma_start(mov[:, 0:MH], mov_d[:, 0:MH])
            nc.sync.dma_start(mov[:, MH:MLOC], mov_d[:, MH:MLOC])

            # PE warm-up + exp-table preload on a never-written (garbage)
            # tile: no data deps, so both start right after the preamble and
            # run while the input DMAs stream; keeps the PE pstate ramp going
            # until the real matmuls have data. Results are never read.
            junk = cpool.tile([P, MH], bf16, tag="junk")
            nc.gpsimd.memset(junk[0:1, 0:1], 0.0)
            for _ in range(5):
                nc.tensor.matmul(
                    wps[:], junk[:, 0:P], junk[:], start=True, stop=True
                )
            scr2 = cpool.tile([1, 8], f32, tag="scr2")
            nc.scalar.activation(scr2[:], junk[0:1, 0:8], EXP)

            ab = cpool.tile([P, 4 * MH], bf16)  # A0 | B0 | A1 | B1 (bf16)
            pp = cpool.tile([P, 2 * MH], bf16)  # PP halves
            sto0 = cpool.tile([2, MH], f32, tag="sto0")
            sto1 = cpool.tile([2, MH], f32, tag="sto1")
            sto = [sto0, sto1]

            # 1) all four E matmuls in one concurrent PE pass
            for h in range(2):
                for ax in range(2):  # 0 = A (axis0), 1 = B (axis1)
                    r = 2 * h + ax
                    nc.tensor.matmul(
                        e[:, r * MH : (r + 1) * MH],
                        stat[32 * r : 32 * r + K, SE0:SW],
                        mov[32 * r : 32 * r + K, h * MH : (h + 1) * MH],
                        start=True,
                        stop=True,
                        tile_position=(32 * r, 0),
                    )
            # 2) exp: B first so the D matmul can start earliest
            for h in range(2):
                ia, ib = 2 * h, 2 * h + 1
                nc.scalar.activation(
                    ab[:, ib * MH : (ib + 1) * MH], e[:, ib * MH : (ib + 1) * MH], EXP
                )
                nc.scalar.activation(
                    ab[:, ia * MH : (ia + 1) * MH], e[:, ia * MH : (ia + 1) * MH], EXP
                )
            for h in range(2):
                ia, ib = 2 * h, 2 * h + 1
                # 3) block-stationary convolution
                nc.tensor.matmul(
                    d[:, h * MH : (h + 1) * MH],
                    stat[:, SD0 : SD0 + P],
                    ab[:, ib * MH : (ib + 1) * MH],
                    start=True,
                    stop=True,
                )
                # 4) PP = A * D
                nc.vector.tensor_mul(
                    pp[:, h * MH : (h + 1) * MH],
                    ab[:, ia * MH : (ia + 1) * MH],
                    d[:, h * MH : (h + 1) * MH],
                )
                # 5) [s;t] = statR^T @ PP
                nc.tensor.matmul(
                    st[32 * h : 32 * h + 2, :],
                    stat[:, SR0 : SR0 + 2],
                    pp[:, h * MH : (h + 1) * MH],
                    start=True,
                    stop=True,
                    tile_position=(0, 32 * h),
                )
                # evict via whichever engine is free by then
                if h == 0:
                    nc.scalar.copy(sto[h][:], st[0:2, :])
                else:
                    nc.vector.tensor_copy(sto[h][:], st[32:34, :])
                nc.sync.dma_start(res_d[2 * h : 2 * h + 2, :], sto[h][:])

    nc.compile()
    return nc


def _bf16_split(v):
    import ml_dtypes

    hi = v.astype(ml_dtypes.bfloat16)
    lo = (v - hi.astype(np.float64)).astype(ml_dtypes.bfloat16)
    return hi, lo


def _prepare(x, inputs, outputs, bandwidth):
    """Host-side O(N + M) prep of grids, splat masses, and coefficients."""
    import ml_dtypes

    x = x.astype(np.float64)
    inputs = inputs.astype(np.float64)
    outputs = outputs.astype(np.float64)
    bw = bandwidth.astype(np.float64)

    # uniform grids + bilinear splat masses
    levels = []
    idx = []
    lam = []
    for ax in range(2):
        vals = inputs[:, ax]
        lv = np.linspace(vals.min(), vals.max(), G)
        j = np.clip(np.searchsorted(lv, vals) - 1, 0, G - 2)
        la = np.clip((vals - lv[j]) / (lv[j + 1] - lv[j]), 0.0, 1.0)
        levels.append(lv)
        idx.append(j)
        lam.append(la)
    (v0, v1), (j0, j1), (l0, l1) = levels, idx, lam
    C = np.zeros((G, G))
    T = np.zeros((G, G))
    for dj in (0, 1):
        for dk in (0, 1):
            wgt = (l0 if dj else 1 - l0) * (l1 if dk else 1 - l1)
            np.add.at(C, (j0 + dj, j1 + dk), wgt)
            np.add.at(T, (j0 + dj, j1 + dk), wgt * outputs)

    stat = np.zeros((P, SW), ml_dtypes.bfloat16)
    # statD block [[C^T, Tlo^T],[0, Thi^T]]
    Ch, _ = _bf16_split(C)
    Th, Tl = _bf16_split(T)
    stat[0:G, SD0 : SD0 + G] = Ch.T
    stat[0:G, SD0 + G : SD0 + P] = Tl.T
    stat[G:P, SD0 + G : SD0 + P] = Th.T
    # statR
    stat[0:G, SR0] = 1.0
    stat[G:P, SR0 + 1] = 1.0
    # statE: rows [1,1,v2h,v2h,v2l,vh,vh,vl] per axis, duplicated across
    # the two 64-col halves; 4 bands (A,B,A,B).
    for ax, v in ((0, v0), (1, v1)):
        vh, vl = _bf16_split(v)
        v2h, v2l = _bf16_split(v * v)
        one = np.ones(G, ml_dtypes.bfloat16)
        band = np.stack([one, one, v2h, v2h, v2l, vh, vh, vl])  # (8, G)
        band = np.concatenate([band, band], axis=1)  # (8, 128) duplicated
        stat[32 * ax : 32 * ax + K, SE0:SW] = band
        stat[32 * (ax + 2) : 32 * (ax + 2) + K, SE0:SW] = band

    # moving rows: [Ph,Pl,Qh,Ql,Qh,Rh,Rl,Rh] per axis, 4 bands (A,B,A,B)
    c = 1.0 / (2.0 * bw * bw)
    Qh, Ql = _bf16_split(-c)
    mov = np.zeros((104, M), ml_dtypes.bfloat16)
    for ax in range(2):
        xc = x[:, ax]
        Ph, Pl = _bf16_split(-c * xc * xc)
        Rh, Rl = _bf16_split(2.0 * c * xc)
        band = np.stack([Ph, Pl, Qh, Ql, Qh, Rh, Rl, Rh])  # (8, M)
        mov[32 * ax : 32 * ax + K, :] = band
        mov[32 * (ax + 2) : 32 * (ax + 2) + K, :] = band

    return stat, mov


def _make_inmaps(x, inputs, outputs, bandwidth):
    stat, mov = _prepare(x, inputs, outputs, bandwidth)
    return [
        {
            "stat": stat,
            "mov": np.ascontiguousarray(mov[:, c * MLOC : (c + 1) * MLOC]),
        }
        for c in range(NCORES)
    ]


def kernel(x, inputs, outputs, bandwidth):
    from concourse.bass_utils import run_bass_kernel_spmd

    x = np.asarray(x, np.float32)
    inputs = np.asarray(inputs, np.float32)
    outputs = np.asarray(outputs, np.float32)
    bandwidth = np.asarray(bandwidth, np.float32)

    in_maps = _make_inmaps(x, inputs, outputs, bandwidth)
    nc = _build()
    try:
        res = run_bass_kernel_spmd(nc, in_maps, list(range(NCORES)))
    except Exception:
        # transient NRT_EXEC_UNIT_UNRECOVERABLE after an interrupted prior
        # run; the device recovers after a short wait.
        import time

        time.sleep(20)
        res = run_bass_kernel_spmd(nc, in_maps, list(range(NCORES)))
    parts = []
    for c in range(NCORES):
        r4 = res.results[c]["res"]  # (4,512): [s_h0; t_h0; s_h1; t_h1]
        s = np.concatenate([r4[0], r4[2]])
        t = np.concatenate([r4[1], r4[3]])
        parts.append(t / (s + EPS))
    return np.concatenate(parts).astype(np.float32)


if __name__ == "__main__":
    rng = np.random.default_rng(0)
    x = rng.standard_normal((M, 2), np.float32)
    inputs = rng.standard_normal((N, 2), np.float32)
    outputs = rng.standard_normal(N, np.float32)
    bandwidth = (0.5 + rng.random(M)).astype(np.float32)
    got = kernel(x, inputs, outputs, bandwidth)
    print(got[:8])


# revision 8
# speedup vs baseline: 4.9156x; 1.0824x over previous
"""Bivariate Gaussian kernel (Nadaraya-Watson) on 8 TRN2 NeuronCores.

Math: for query m, result[m] = t[m] / (s[m] + EPS) where
  w[n,m] = exp(-c[m] * d2[n,m]),  c[m] = 1/(2*bw[m]^2)
  s[m] = sum_n w[n,m],  t[m] = sum_n w[n,m]*outputs[n]

Algorithm (separable quantized convolution, a fast-Gauss-transform):
the 2-D Gaussian factorizes per coordinate,
  w[n,m] = exp(-c*(i0[n]-x0[m])^2) * exp(-c*(i1[n]-x1[m])^2).
Each input coordinate is splat with linear-interpolation weights onto a
uniform g-level grid (host side, O(N)), giving grid mass C[j,k] and
output-weighted mass T[j,k].  Then
  s[m] = sum_jk C[j,k] * A[j,m] * B[k,m],   A[j,m]=exp(-c[m](v0_j-x0[m])^2)
  t[m] = sum_jk T[j,k] * A[j,m] * B[k,m],   B[k,m]=exp(-c[m](v1_k-x1[m])^2)
Bilinear splatting preserves the linear term of the exponent exactly, so
the error is O(spacing^2); g=64 gives rel err ~5e-3 (tolerance 2e-2).

Device flow per core (MLOC=1024 queries, g=64):
  1. E tables as rank-8 error-compensated bf16 matmuls
     E_A[j,m] = P0[m] + R0[m] v0_j + Q[m] v0_j^2  (and E_B with axis 1),
     content duplicated across two 64-column blocks so each table lives
     on all 128 partitions; two strips (A at array rows 0-7, B at rows
     32-39) run concurrently via tile_position.
  2. ACT: Abuf = exp(E_A), Bbuf = exp(E_B)  (PSUM f32 -> SBUF bf16).
  3. One 128x128 block stationary [[C^T, Tlo^T],[0, Thi^T]] x Bbuf
     computes D = [D_s (rows 0-63); D_t (rows 64-127)] in one pass.
  4. DVE: PP = Abuf * D  (bf16).
  5. Reduce stationary (128,2) [[1,0],[0,1]] blocks -> [s; t] per m-half.
Host: r = t / (s + EPS).  Queries (M) sharded across the 8 cores.
"""

import functools
import sys

import numpy as np

sys.path.insert(0, "/opt/trn_rl_repo")

EPS = 1e-7
N = 8192
M = 8192
NCORES = 8
MLOC = M // NCORES  # 1024
P = 128
G = 64  # grid levels per axis
MH = 512  # m-half width (one PSUM bank)
K = 8  # compensated-split rank
# packed stat tensor columns: statD | statR | statE
SD0, SR0, SE0, SW = 0, P, P + 2, P + 2 + P


@functools.lru_cache(maxsize=1)
def _build():
    import concourse.tile as tile
    from concourse import bacc, mybir

    f32 = mybir.dt.float32
    bf16 = mybir.dt.bfloat16
    EXP = mybir.ActivationFunctionType.Exp

    nc = bacc.Bacc("TRN2", target_bir_lowering=False, debug=False, num_devices=NCORES)
    # packed stationaries (one DMA): cols 0-127 = statD (the block
    # stationary [[C^T, Tlo^T],[0, Thi^T]]), cols 128-129 = statR (reduce:
    # col 0 = ones on rows 0-63, col 1 = ones on rows 64-127), cols
    # 130-257 = statE (2 bands of 8 rows at partition offsets 0/32 (A, B);
    # each band holds the axis' hi/lo-split level rows duplicated across
    # the two 64-col halves).
    stat_d = nc.dram_tensor("stat", [P, SW], bf16, kind="ExternalInput")
    # moving per-m coefficient rows, same 2-band layout as statE
    mov_d = nc.dram_tensor("mov", [40, MLOC], bf16, kind="ExternalInput")
    res_d = nc.dram_tensor("res", [4, MH], f32, kind="ExternalOutput")

    with tile.TileContext(nc) as tc:
        with (
            tc.tile_pool(name="const", bufs=1) as cpool,
            tc.tile_pool(name="psum", bufs=1, space="PSUM") as ppool,
        ):
            # one PSUM bank per logical tile so cross-engine dependencies
            # stay per-tile (a shared multi-bank tile serializes readers
            # behind every writer of the tile)
            ea0 = ppool.tile([P, MH], f32)
            eb0 = ppool.tile([P, MH], f32)
            ea1 = ppool.tile([P, MH], f32)
            eb1 = ppool.tile([P, MH], f32)
            d0 = ppool.tile([P, MH], f32)
            d1 = ppool.tile([P, MH], f32)
            st = ppool.tile([34, MH], f32)  # [s;t] at rows 0-1 (h0), 32-33 (h1)
            wps = ppool.tile([P, MH], f32)  # warm-up target bank

            # input loads first so the HWDGE queues start immediately
            stat = cpool.tile([P, SW], bf16)
            mov = cpool.tile([40, MLOC], bf16)
            nc.sync.dma_start(stat[:], stat_d[:])
            nc.scalar.dma_start(mov[:], mov_d[:])

            # PE warm-up + exp-table preload on a never-written (garbage)
            # tile: no data deps, so both start right after the preamble and
            # run while the input DMAs stream; keeps the PE pstate ramp going
            # until the real matmuls have data. Results are never read.
            junk = cpool.tile([P, MH], bf16, tag="junk")
            nc.gpsimd.memset(junk[0:1, 0:1], 0.0)
            for _ in range(3):
                nc.tensor.matmul(
                    wps[:], junk[:, 0:P], junk[:], start=True, stop=True
                )
            scr2 = cpool.tile([1, 8], f32, tag="scr2")
            nc.scalar.activation(scr2[:], junk[0:1, 0:8], EXP)

            ab = cpool.tile([P, 4 * MH], bf16)  # A0 | B0 | A1 | B1 (bf16)
            pp = cpool.tile([P, 2 * MH], bf16)  # PP halves
            sto = cpool.tile([34, MH], f32)

            # 1) E matmuls: strips A (rows 0-7) and B (rows 32-39) run
            # concurrently; h=0/1 sequential within each strip
            et = [[ea0, eb0], [ea1, eb1]]
            for h in range(2):
                for ax in range(2):  # 0 = A (axis0), 1 = B (axis1)
                    nc.tensor.matmul(
                        et[h][ax][:],
                        stat[32 * ax : 32 * ax + K, SE0:SW],
                        mov[32 * ax : 32 * ax + K, h * MH : (h + 1) * MH],
                        start=True,
                        stop=True,
                        tile_position=(32 * ax, 0),
                    )
            # 2) exp: B first so the D matmul can start earliest
            for h in range(2):
                ia, ib = 2 * h, 2 * h + 1
                nc.scalar.activation(ab[:, ib * MH : (ib + 1) * MH], et[h][1][:], EXP)
                nc.scalar.activation(ab[:, ia * MH : (ia + 1) * MH], et[h][0][:], EXP)
            # 3) block-stationary convolutions (PE order: D0, D1, R0, R1 so
            # D1 isn't stuck behind the mul0-dependent R0)
            for h, dt_ in ((0, d0), (1, d1)):
                ib = 2 * h + 1
                nc.tensor.matmul(
                    dt_[:],
                    stat[:, SD0 : SD0 + P],
                    ab[:, ib * MH : (ib + 1) * MH],
                    start=True,
                    stop=True,
                )
            # 4) PP = A * D on DVE
            for h, dt_ in ((0, d0), (1, d1)):
                ia = 2 * h
                nc.vector.tensor_mul(
                    pp[:, h * MH : (h + 1) * MH],
                    ab[:, ia * MH : (ia + 1) * MH],
                    dt_[:],
                )
            # 5) [s;t] = statR^T @ PP
            for h in range(2):
                nc.tensor.matmul(
                    st[32 * h : 32 * h + 2, :],
                    stat[:, SR0 : SR0 + 2],
                    pp[:, h * MH : (h + 1) * MH],
                    start=True,
                    stop=True,
                    tile_position=(0, 32 * h),
                )
            # evict: h0 via scalar (free after the exps), h1 via vector
            nc.scalar.copy(sto[0:2, :], st[0:2, :])
            nc.sync.dma_start(res_d[0:2, :], sto[0:2, :])
            nc.vector.tensor_copy(sto[32:34, :], st[32:34, :])
            nc.sync.dma_start(res_d[2:4, :], sto[32:34, :])

    nc.compile()
    return nc


def _bf16_split(v):
    import ml_dtypes

    hi = v.astype(ml_dtypes.bfloat16)
    lo = (v - hi.astype(np.float64)).astype(ml_dtypes.bfloat16)
    return hi, lo


def _prepare(x, inputs, outputs, bandwidth):
    """Host-side O(N + M) prep of grids, splat masses, and coefficients."""
    import ml_dtypes

    x = x.astype(np.float64)
    inputs = inputs.astype(np.float64)
    outputs = outputs.astype(np.float64)
    bw = bandwidth.astype(np.float64)

    # uniform grids + bilinear splat masses
    levels = []
    idx = []
    lam = []
    for ax in range(2):
        vals = inputs[:, ax]
        lv = np.linspace(vals.min(), vals.max(), G)
        j = np.clip(np.searchsorted(lv, vals) - 1, 0, G - 2)
        la = np.clip((vals - lv[j]) / (lv[j + 1] - lv[j]), 0.0, 1.0)
        levels.append(lv)
        idx.append(j)
        lam.append(la)
    (v0, v1), (j0, j1), (l0, l1) = levels, idx, lam
    C = np.zeros((G, G))
    T = np.zeros((G, G))
    for dj in (0, 1):
        for dk in (0, 1):
            wgt = (l0 if dj else 1 - l0) * (l1 if dk else 1 - l1)
            np.add.at(C, (j0 + dj, j1 + dk), wgt)
            np.add.at(T, (j0 + dj, j1 + dk), wgt * outputs)

    stat = np.zeros((P, SW), ml_dtypes.bfloat16)
    # statD block [[C^T, Tlo^T],[0, Thi^T]]
    Ch, _ = _bf16_split(C)
    Th, Tl = _bf16_split(T)
    stat[0:G, SD0 : SD0 + G] = Ch.T
    stat[0:G, SD0 + G : SD0 + P] = Tl.T
    stat[G:P, SD0 + G : SD0 + P] = Th.T
    # statR
    stat[0:G, SR0] = 1.0
    stat[G:P, SR0 + 1] = 1.0
    # statE: rows [1,1,v2h,v2h,v2l,vh,vh,vl] per axis, duplicated across
    # the two 64-col halves; 2 bands (A at partitions 0-7, B at 32-39).
    for ax, v in ((0, v0), (1, v1)):
        vh, vl = _bf16_split(v)
        v2h, v2l = _bf16_split(v * v)
        one = np.ones(G, ml_dtypes.bfloat16)
        band = np.stack([one, one, v2h, v2h, v2l, vh, vh, vl])  # (8, G)
        band = np.concatenate([band, band], axis=1)  # (8, 128) duplicated
        stat[32 * ax : 32 * ax + K, SE0:SW] = band

    # moving rows: [Ph,Pl,Qh,Ql,Qh,Rh,Rl,Rh] per axis, 2 bands (A, B)
    c = 1.0 / (2.0 * bw * bw)
    Qh, Ql = _bf16_split(-c)
    mov = np.zeros((40, M), ml_dtypes.bfloat16)
    for ax in range(2):
        xc = x[:, ax]
        Ph, Pl = _bf16_split(-c * xc * xc)
        Rh, Rl = _bf16_split(2.0 * c * xc)
        band = np.stack([Ph, Pl, Qh, Ql, Qh, Rh, Rl, Rh])  # (8, M)
        mov[32 * ax : 32 * ax + K, :] = band

    return stat, mov


def _make_inmaps(x, inputs, outputs, bandwidth):
    stat, mov = _prepare(x, inputs, outputs, bandwidth)
    return [
        {
            "stat": stat,
            "mov": np.ascontiguousarray(mov[:, c * MLOC : (c + 1) * MLOC]),
        }
        for c in range(NCORES)
    ]


def kernel(x, inputs, outputs, bandwidth):
    from concourse.bass_utils import run_bass_kernel_spmd

    x = np.asarray(x, np.float32)
    inputs = np.asarray(inputs, np.float32)
    outputs = np.asarray(outputs, np.float32)
    bandwidth = np.asarray(bandwidth, np.float32)

    in_maps = _make_inmaps(x, inputs, outputs, bandwidth)
    nc = _build()
    try:
        res = run_bass_kernel_spmd(nc, in_maps, list(range(NCORES)))
    except Exception:
        # transient NRT_EXEC_UNIT_UNRECOVERABLE after an interrupted prior
        # run; the device recovers after a short wait.
        import time

        time.sleep(20)
        res = run_bass_kernel_spmd(nc, in_maps, list(range(NCORES)))
    parts = []
    for c in range(NCORES):
        r4 = res.results[c]["res"]  # (4,512): [s_h0; t_h0; s_h1; t_h1]
        s = np.concatenate([r4[0], r4[2]])
        t = np.concatenate([r4[1], r4[3]])
        parts.append(t / (s + EPS))
    return np.concatenate(parts).astype(np.float32)


if __name__ == "__main__":
    rng = np.random.default_rng(0)
    x = rng.standard_normal((M, 2), np.float32)
    inputs = rng.standard_normal((N, 2), np.float32)
    outputs = rng.standard_normal(N, np.float32)
    bandwidth = (0.5 + rng.random(M)).astype(np.float32)
    got = kernel(x, inputs, outputs, bandwidth)
    print(got[:8])


# revision 9
# speedup vs baseline: 5.0082x; 1.0188x over previous
"""Bivariate Gaussian kernel (Nadaraya-Watson) on 8 TRN2 NeuronCores.

Math: for query m, result[m] = t[m] / (s[m] + EPS) where
  w[n,m] = exp(-c[m] * d2[n,m]),  c[m] = 1/(2*bw[m]^2)
  s[m] = sum_n w[n,m],  t[m] = sum_n w[n,m]*outputs[n]

Algorithm (separable quantized convolution, a fast-Gauss-transform):
the 2-D Gaussian factorizes per coordinate,
  w[n,m] = exp(-c*(i0[n]-x0[m])^2) * exp(-c*(i1[n]-x1[m])^2).
Each input coordinate is splat with linear-interpolation weights onto a
uniform g-level grid (host side, O(N)), giving grid mass C[j,k] and
output-weighted mass T[j,k].  Then
  s[m] = sum_jk C[j,k] * A[j,m] * B[k,m],   A[j,m]=exp(-c[m](v0_j-x0[m])^2)
  t[m] = sum_jk T[j,k] * A[j,m] * B[k,m],   B[k,m]=exp(-c[m](v1_k-x1[m])^2)
Bilinear splatting preserves the linear term of the exponent exactly, so
the error is O(spacing^2); g=64 gives rel err ~5e-3 (tolerance 2e-2).

Device flow per core (MLOC=1024 queries, g=64):
  1. E tables as rank-8 error-compensated bf16 matmuls
     E_A[j,m] = P0[m] + R0[m] v0_j + Q[m] v0_j^2  (and E_B with axis 1),
     content duplicated across two 64-column blocks so each table lives
     on all 128 partitions; two strips (A at array rows 0-7, B at rows
     32-39) run concurrently via tile_position.
  2. ACT: Abuf = exp(E_A), Bbuf = exp(E_B)  (PSUM f32 -> SBUF bf16).
  3. One 128x128 block stationary [[C^T, Tlo^T],[0, Thi^T]] x Bbuf
     computes D = [D_s (rows 0-63); D_t (rows 64-127)] in one pass.
  4. DVE: PP = Abuf * D  (bf16).
  5. Reduce stationary (128,2) [[1,0],[0,1]] blocks -> [s; t] per m-half.
Host: r = t / (s + EPS).  Queries (M) sharded across the 8 cores.
"""

import functools
import sys

import numpy as np

sys.path.insert(0, "/opt/trn_rl_repo")

EPS = 1e-7
N = 8192
M = 8192
NCORES = 8
MLOC = M // NCORES  # 1024
P = 128
G = 64  # grid levels per axis
MH = 512  # m-half width (one PSUM bank)
K = 8  # compensated-split rank
# packed stat tensor columns: statD | statR | statE
SD0, SR0, SE0, SW = 0, P, P + 2, P + 2 + P


@functools.lru_cache(maxsize=1)
def _build():
    import concourse.tile as tile
    from concourse import bacc, mybir

    f32 = mybir.dt.float32
    bf16 = mybir.dt.bfloat16
    EXP = mybir.ActivationFunctionType.Exp

    nc = bacc.Bacc("TRN2", target_bir_lowering=False, debug=False, num_devices=NCORES)
    # packed stationaries: cols 0-127 = statD (the block stationary
    # [[C^T, Tlo^T],[0, Thi^T]]), cols 128-129 = statR (reduce: col 0 =
    # ones on rows 0-63, col 1 = ones on rows 64-127), cols 130-257 =
    # statE (2 bands of 8 rows at partition offsets 0/32 (A, B); each
    # band holds the axis' hi/lo-split level rows duplicated across the
    # two 64-col halves).  DMA'd in pieces so the E tables don't wait on
    # the statD bytes.
    stat_d = nc.dram_tensor("stat", [P, SW], bf16, kind="ExternalInput")
    # moving per-m coefficient rows, same 2-band layout as statE
    mov_d = nc.dram_tensor("mov", [40, MLOC], bf16, kind="ExternalInput")
    res_d = nc.dram_tensor("res", [4, MH], f32, kind="ExternalOutput")

    with tile.TileContext(nc) as tc:
        with (
            tc.tile_pool(name="const", bufs=1) as cpool,
            tc.tile_pool(name="psum", bufs=1, space="PSUM") as ppool,
        ):
            # one PSUM bank per logical tile so cross-engine dependencies
            # stay per-tile (a shared multi-bank tile serializes readers
            # behind every writer of the tile)
            ea0 = ppool.tile([P, MH], f32)
            eb0 = ppool.tile([P, MH], f32)
            ea1 = ppool.tile([P, MH], f32)
            eb1 = ppool.tile([P, MH], f32)
            d0 = ppool.tile([P, MH], f32)
            d1 = ppool.tile([P, MH], f32)
            st0 = ppool.tile([2, MH], f32)
            st1 = ppool.tile([2, MH], f32)

            # input loads first so the HWDGE queues start immediately; the
            # E tables need statE + mov, so those go first on both queues
            statE = cpool.tile([40, P], bf16)
            statDR = cpool.tile([P, P + 2], bf16)
            mov = cpool.tile([40, MLOC], bf16)
            nc.sync.dma_start(statE[:], stat_d[0:40, SE0:SW])
            nc.scalar.dma_start(mov[:, 0:MH], mov_d[:, 0:MH])
            nc.sync.dma_start(mov[:, MH:MLOC], mov_d[:, MH:MLOC])
            nc.sync.dma_start(statDR[:], stat_d[:, 0:SE0])

            # PE warm-up + exp-table preload on a never-written (garbage)
            # tile: no data deps, so both start right after the preamble and
            # run while the input DMAs stream; keeps the PE pstate ramp going
            # until the real matmuls have data. Results are never read (the
            # warm-ups land in d0, which D0 later overwrites with start=True).
            junk = cpool.tile([P, MH], bf16, tag="junk")
            nc.gpsimd.memset(junk[0:1, 0:1], 0.0)
            for _ in range(3):
                nc.tensor.matmul(
                    d0[:], junk[:, 0:P], junk[:], start=True, stop=True
                )
            scr2 = cpool.tile([1, 8], f32, tag="scr2")
            nc.scalar.activation(scr2[:], junk[0:1, 0:8], EXP)

            ab = cpool.tile([P, 4 * MH], bf16)  # A0 | B0 | A1 | B1 (bf16)
            pp = cpool.tile([P, 2 * MH], bf16)  # PP halves
            sto0 = cpool.tile([2, MH], f32, tag="sto0")
            sto1 = cpool.tile([2, MH], f32, tag="sto1")

            # 1) E matmuls: strips A (rows 0-7) and B (rows 32-39) run
            # concurrently; h=0/1 sequential within each strip
            et = [[ea0, eb0], [ea1, eb1]]
            for h in range(2):
                for ax in range(2):  # 0 = A (axis0), 1 = B (axis1)
                    nc.tensor.matmul(
                        et[h][ax][:],
                        statE[32 * ax : 32 * ax + K, :],
                        mov[32 * ax : 32 * ax + K, h * MH : (h + 1) * MH],
                        start=True,
                        stop=True,
                        tile_position=(32 * ax, 0),
                    )
            # 2) exp: B first so the D matmul can start earliest
            for h in range(2):
                ia, ib = 2 * h, 2 * h + 1
                nc.scalar.activation(ab[:, ib * MH : (ib + 1) * MH], et[h][1][:], EXP)
                nc.scalar.activation(ab[:, ia * MH : (ia + 1) * MH], et[h][0][:], EXP)
            # 3) block-stationary convolutions (PE order: D0, D1, R0, R1 so
            # D1 isn't stuck behind the mul0-dependent R0)
            for h, dt_ in ((0, d0), (1, d1)):
                ib = 2 * h + 1
                nc.tensor.matmul(
                    dt_[:],
                    statDR[:, 0:P],
                    ab[:, ib * MH : (ib + 1) * MH],
                    start=True,
                    stop=True,
                )
            # 4) PP = A * D on DVE
            for h, dt_ in ((0, d0), (1, d1)):
                ia = 2 * h
                nc.vector.tensor_mul(
                    pp[:, h * MH : (h + 1) * MH],
                    ab[:, ia * MH : (ia + 1) * MH],
                    dt_[:],
                )
            # 5) [s;t] = statR^T @ PP
            for h, st_ in ((0, st0), (1, st1)):
                nc.tensor.matmul(
                    st_[:],
                    statDR[:, P : P + 2],
                    pp[:, h * MH : (h + 1) * MH],
                    start=True,
                    stop=True,
                )
            # evict: h0 via scalar (free after the exps), h1 via vector
            nc.scalar.copy(sto0[:], st0[:])
            nc.sync.dma_start(res_d[0:2, :], sto0[:])
            nc.vector.tensor_copy(sto1[:], st1[:])
            nc.sync.dma_start(res_d[2:4, :], sto1[:])

    nc.compile()
    return nc


def _bf16_split(v):
    import ml_dtypes

    hi = v.astype(ml_dtypes.bfloat16)
    lo = (v - hi.astype(np.float64)).astype(ml_dtypes.bfloat16)
    return hi, lo


def _prepare(x, inputs, outputs, bandwidth):
    """Host-side O(N + M) prep of grids, splat masses, and coefficients."""
    import ml_dtypes

    x = x.astype(np.float64)
    inputs = inputs.astype(np.float64)
    outputs = outputs.astype(np.float64)
    bw = bandwidth.astype(np.float64)

    # uniform grids + bilinear splat masses
    levels = []
    idx = []
    lam = []
    for ax in range(2):
        vals = inputs[:, ax]
        lv = np.linspace(vals.min(), vals.max(), G)
        j = np.clip(np.searchsorted(lv, vals) - 1, 0, G - 2)
        la = np.clip((vals - lv[j]) / (lv[j + 1] - lv[j]), 0.0, 1.0)
        levels.append(lv)
        idx.append(j)
        lam.append(la)
    (v0, v1), (j0, j1), (l0, l1) = levels, idx, lam
    C = np.zeros((G, G))
    T = np.zeros((G, G))
    for dj in (0, 1):
        for dk in (0, 1):
            wgt = (l0 if dj else 1 - l0) * (l1 if dk else 1 - l1)
            np.add.at(C, (j0 + dj, j1 + dk), wgt)
            np.add.at(T, (j0 + dj, j1 + dk), wgt * outputs)

    stat = np.zeros((P, SW), ml_dtypes.bfloat16)
    # statD block [[C^T, Tlo^T],[0, Thi^T]]
    Ch, _ = _bf16_split(C)
    Th, Tl = _bf16_split(T)
    stat[0:G, SD0 : SD0 + G] = Ch.T
    stat[0:G, SD0 + G : SD0 + P] = Tl.T
    stat[G:P, SD0 + G : SD0 + P] = Th.T
    # statR
    stat[0:G, SR0] = 1.0
    stat[G:P, SR0 + 1] = 1.0
    # statE: rows [1,1,v2h,v2h,v2l,vh,vh,vl] per axis, duplicated across
    # the two 64-col halves; 2 bands (A at partitions 0-7, B at 32-39).
    for ax, v in ((0, v0), (1, v1)):
        vh, vl = _bf16_split(v)
        v2h, v2l = _bf16_split(v * v)
        one = np.ones(G, ml_dtypes.bfloat16)
        band = np.stack([one, one, v2h, v2h, v2l, vh, vh, vl])  # (8, G)
        band = np.concatenate([band, band], axis=1)  # (8, 128) duplicated
        stat[32 * ax : 32 * ax + K, SE0:SW] = band

    # moving rows: [Ph,Pl,Qh,Ql,Qh,Rh,Rl,Rh] per axis, 2 bands (A, B)
    c = 1.0 / (2.0 * bw * bw)
    Qh, Ql = _bf16_split(-c)
    mov = np.zeros((40, M), ml_dtypes.bfloat16)
    for ax in range(2):
        xc = x[:, ax]
        Ph, Pl = _bf16_split(-c * xc * xc)
        Rh, Rl = _bf16_split(2.0 * c * xc)
        band = np.stack([Ph, Pl, Qh, Ql, Qh, Rh, Rl, Rh])  # (8, M)
        mov[32 * ax : 32 * ax + K, :] = band

    return stat, mov


def _make_inmaps(x, inputs, outputs, bandwidth):
    stat, mov = _prepare(x, inputs, outputs, bandwidth)
    return [
        {
            "stat": stat,
            "mov": np.ascontiguousarray(mov[:, c * MLOC : (c + 1) * MLOC]),
        }
        for c in range(NCORES)
    ]


def kernel(x, inputs, outputs, bandwidth):
    from concourse.bass_utils import run_bass_kernel_spmd

    x = np.asarray(x, np.float32)
    inputs = np.asarray(inputs, np.float32)
    outputs = np.asarray(outputs, np.float32)
    bandwidth = np.asarray(bandwidth, np.float32)

    in_maps = _make_inmaps(x, inputs, outputs, bandwidth)
    nc = _build()
    try:
        res = run_bass_kernel_spmd(nc, in_maps, list(range(NCORES)))
    except Exception:
        # transient NRT_EXEC_UNIT_UNRECOVERABLE after an interrupted prior
        # run; the device recovers after a short wait.
        import time

        time.sleep(20)
        res = run_bass_kernel_spmd(nc, in_maps, list(range(NCORES)))
    parts = []
    for c in range(NCORES):
        r4 = res.results[c]["res"]  # (4,512): [s_h0; t_h0; s_h1; t_h1]
        s = np.concatenate([r4[0], r4[2]])
        t = np.concatenate([r4[1], r4[3]])
        parts.append(t / (s + EPS))
    return np.concatenate(parts).astype(np.float32)


if __name__ == "__main__":
    rng = np.random.default_rng(0)
    x = rng.standard_normal((M, 2), np.float32)
    inputs = rng.standard_normal((N, 2), np.float32)
    outputs = rng.standard_normal(N, np.float32)
    bandwidth = (0.5 + rng.random(M)).astype(np.float32)
    got = kernel(x, inputs, outputs, bandwidth)
    print(got[:8])
